# revision 10
# baseline (speedup 1.0000x reference)
"""Trainium2 Bass kernel for the DinMod LSTM+CfC (NCP) recurrent network.

Parallel-in-time Picard iteration. The graded execute path costs ~45us
per emitted instruction (fully serialized, size-independent), so instead
of an exact T=512 sequential scan (~16k instructions), we iterate the
whole trajectory: guess h[t]=0, then each sweep recomputes all T steps
with BATCHED instructions (matmuls/ACT/DVE over 4 sequences x 512 steps
at once). The LSTM c-recurrence is linear given the gates, so ONE
tensor_tensor_scan instruction solves it exactly along the time axis per
sweep. The step map is a strong contraction (~10x error reduction per
sweep, measured on the actual weights): 6 sweeps reach ~3e-6 relative
error in fp64 (tolerance is 2e-2).

Per core: 8 sequences (batch elems), processed as 2 independent halves
of 4 (SBUF budget). Column layout per half: 4 segments of 513 columns
(1 zero pad + 512 steps); col(s, t) = s*513 + 1 + t. Trajectory tiles
read at offset 0 give h[t-1] (shifted), offset 1 gives h[t]; the pad
column doubles as the zero initial state and as the scan reset (the
sfg multiplier and S2 addend are 0 there, so the running c state resets
across segment boundaries in the single flattened scan).

Per sweep per half (~65 instructions):
  gates: 4 segs x (inject zinA | whA@h | inject zinB | whB@h) = 16 mm
         -> sigmoid(fg+1)/sigmoid(og) (1 ACT over both groups),
            sigmoid(ig), tanh(ia) [+1 folded into the zinA bias]
  c:     S2 = sig*tia (DVE), c = tensor_tensor_scan(sfg, S2),
         tanh(c) (ACT), h_lstm = tc*sog (DVE)
  CfC l: 2 mm/seg (inject or h0/h1-part + recurrent part), 3 ACT
         (tanh f1, tanh f2, sigmoid ti), 3 DVE (D=f2-f1, G=ti*D,
         h_l = f1+G -> written time-shifted into the H tile)

Verifier constraints honored: all access patterns start at partition
0/32/64/96; dual-SBUF vector operands have equal base partitions
(mixed SBUF/PSUM exempt); PSUM accumulation groups are start/stop pairs
within one bank.
"""

import numpy as np

import concourse.bass as bass
import concourse.mybir as mybir
from concourse import bacc
from concourse.tile import TileContext
from concourse.bass_utils import run_bass_kernel_spmd

IN_DIM, LATENT = 512, 256
INTER, COMMAND, MOTOR = 18, 12, 3
STATE = INTER + COMMAND + MOTOR  # 33
B, T_FULL, N_CORES = 64, 512, 8
BS = B // N_CORES  # 8 sequences per core
HB = 4             # sequences per half
SEG = T_FULL + 1   # 513 padded columns per sequence
NH = HB * SEG      # 2052 columns per half
NSWEEPS = 4

F32 = mybir.dt.float32
AF = mybir.ActivationFunctionType
ALU = mybir.AluOpType

# ---------------------------------------------------------------------------
# Weight blob: every lhsT/bias lives at rows 0:r, cols off:off+c of a single
# [128, BW] tensor (one DMA; base-0 quadrant-legal slices).
# ---------------------------------------------------------------------------
# (name, rows, cols, base_row): base_row shifts the block down so the
# lhsT slice's base partition matches its rhs (matmul requires equality).
_BLOCKS = [
    ("I97", 97, 97, 0), ("I82", 82, 82, 0),
    ("whA", 67, 97, 0), ("whB", 67, 97, 0),
    ("L0m", 114, 82, 0), ("L1m", 128, 76, 0),
    ("P2h1", 12, 67, 32), ("P2h", 33, 67, 0),
    ("bA", 97, 1, 0), ("bB", 97, 1, 0), ("bC", 82, 1, 0),
    ("b1", 76, 1, 0), ("b2", 67, 1, 0),
] + [(f"pa{g}{k}", 128, 97 if g in "AB" else 82, 0)
     for g in "ABC" for k in range(4)]

_OFFS = {}
_BW = 0
for _nm, _r, _c, _b in _BLOCKS:
    _OFFS[_nm] = (_r, _c, _BW, _b)
    _BW += _c

# HT (h trajectory) row layout: h0@0:18, h1@32:44, h2@64:67 (quadrant-legal
# write starts for the three per-layer h writers); unused rows stay 0.
_HTROW = [j for j in range(18)] + [32 + j for j in range(12)] + [64 + j for j in range(3)]


def prep_weights(inp):
    g = {k: np.asarray(v, np.float64) for k, v in inp.items()}
    Wf, bf = g["fc1_w"], g["fc1_b"]            # (256,512), (256,)
    wi, bi, wh = g["lstm_wi"], g["lstm_bi"], g["lstm_wh"]
    ia, ig, fg, og = (slice(0, 33), slice(33, 66),
                      slice(66, 99), slice(99, 132))

    blob = np.zeros((128, _BW), np.float64)

    def put(nm, m):
        r, c, off, b = _OFFS[nm]
        assert m.shape == (r, c), (nm, m.shape, (r, c))
        blob[b:b + r, off:off + c] = m

    put("I97", np.eye(97))
    put("I82", np.eye(82))

    # recurrent gate contractions: out rows (grp A) fg@0:33, ig@64:97;
    # (grp B) og@0:33, ia@64:97. Contract dim = HT rows (h comp j at _HTROW[j]).
    def wh_block(lo_sl, hi_sl):
        m = np.zeros((67, 97))
        for j in range(STATE):
            r = _HTROW[j]
            m[r, 0:33] = wh[lo_sl, j]
            m[r, 64:97] = wh[hi_sl, j]
        return m

    put("whA", wh_block(fg, ig))
    put("whB", wh_block(og, ia))

    # CfC layer weights (masked), ti = sigmoid((ta+tb) @ xc + (tab+tbb))
    w1m, w2m, wab, bab = [], [], [], []
    for l in range(3):
        w1m.append(g[f"ff1w{l}"] * g[f"mask{l}"])
        w2m.append(g[f"ff2w{l}"] * g[f"mask{l}"])
        wab.append(g[f"taw{l}"] + g[f"tbw{l}"])
        bab.append(g[f"tab{l}"] + g[f"tbb{l}"])

    # layer 0, one matmul: rhs = ZC (zinC@0:82, hs0 copy@96:114); out rows
    # f1@0:18, f2@32:50, ti@64:82
    m = np.zeros((114, 82))
    m[0:82, 0:82] = np.eye(82)
    for j in range(INTER):
        m[96 + j, 0:18] = w1m[0][:, LATENT + j]
        m[96 + j, 32:50] = w2m[0][:, LATENT + j]
        m[96 + j, 64:82] = wab[0][:, LATENT + j]
    put("L0m", m)

    # layer 1, one matmul: rhs = HT (h0@0:18, hl copy@96:128 -> hs1@114:126);
    # out rows f1@0:12, f2@32:44, ti@64:76
    m = np.zeros((128, 76))
    for j in range(INTER):
        m[j, 0:12] = w1m[1][:, j]
        m[j, 32:44] = w2m[1][:, j]
        m[j, 64:76] = wab[1][:, j]
    for j in range(COMMAND):
        m[114 + j, 0:12] = w1m[1][:, INTER + j]
        m[114 + j, 32:44] = w2m[1][:, INTER + j]
        m[114 + j, 64:76] = wab[1][:, INTER + j]
    put("L1m", m)
    bs1 = np.zeros((76, 1))
    bs1[0:12, 0] = g["ff1b1"]
    bs1[32:44, 0] = g["ff2b1"]
    bs1[64:76, 0] = bab[1]
    put("b1", bs1)

    # layer 2: xc = [h1(12), hs2(3)]; out rows f1@0:3, f2@32:35, ti@64:67
    m = np.zeros((12, 67))
    for j in range(COMMAND):
        m[j, 0:3] = w1m[2][:, j]
        m[j, 32:35] = w2m[2][:, j]
        m[j, 64:67] = wab[2][:, j]
    put("P2h1", m)
    m = np.zeros((33, 67))
    for j in range(MOTOR):
        m[30 + j, 0:3] = w1m[2][:, COMMAND + j]
        m[30 + j, 32:35] = w2m[2][:, COMMAND + j]
        m[30 + j, 64:67] = wab[2][:, COMMAND + j]
    put("P2h", m)
    bs2 = np.zeros((67, 1))
    bs2[0:3, 0] = g["ff1b2"]
    bs2[32:35, 0] = g["ff2b2"]
    bs2[64:67, 0] = bab[2]
    put("b2", bs2)

    # phase A composed input projections (fc1 folded in); +1.0 on the fg
    # bias so the sigmoid ACT needs no extra bias.
    WA = np.zeros((97, IN_DIM)); bA = np.zeros((97, 1))
    WA[0:33] = wi[fg] @ Wf
    bA[0:33, 0] = wi[fg] @ bf + bi[fg] + 1.0
    WA[64:97] = wi[ig] @ Wf
    bA[64:97, 0] = wi[ig] @ bf + bi[ig]
    WB = np.zeros((97, IN_DIM)); bB = np.zeros((97, 1))
    WB[0:33] = wi[og] @ Wf
    bB[0:33, 0] = wi[og] @ bf + bi[og]
    WB[64:97] = wi[ia] @ Wf
    bB[64:97, 0] = wi[ia] @ bf + bi[ia]
    WC = np.zeros((82, IN_DIM)); bC = np.zeros((82, 1))
    WC[0:18] = w1m[0][:, 0:LATENT] @ Wf
    bC[0:18, 0] = w1m[0][:, 0:LATENT] @ bf + g["ff1b0"]
    WC[32:50] = w2m[0][:, 0:LATENT] @ Wf
    bC[32:50, 0] = w2m[0][:, 0:LATENT] @ bf + g["ff2b0"]
    WC[64:82] = wab[0][:, 0:LATENT] @ Wf
    bC[64:82, 0] = wab[0][:, 0:LATENT] @ bf + bab[0]
    put("bA", bA)
    put("bB", bB)
    put("bC", bC)
    for gname, W in (("A", WA), ("B", WB), ("C", WC)):
        for k in range(4):
            put(f"pa{gname}{k}",
                np.ascontiguousarray(W[:, 128 * k:128 * (k + 1)].T))

    return {"wblob": blob.astype(np.float32)}


def build_program(T=T_FULL, opts=()):
    assert T == T_FULL
    opts = set(opts)
    sweep_reps = 1
    for o in opts:
        if isinstance(o, str) and o.startswith("reps"):
            sweep_reps = int(o[4:])

    nc = bacc.Bacc("TRN2")
    xt_d = nc.dram_tensor("xt", [IN_DIM, BS * T], F32, kind="ExternalInput")
    wb_d = nc.dram_tensor("wblob", [128, _BW], F32, kind="ExternalInput")
    out_d = nc.dram_tensor("out", [MOTOR, BS, T], F32, kind="ExternalOutput")

    with TileContext(nc) as tc:
        with tc.tile_pool(name="wpool", bufs=1) as wp, \
             tc.tile_pool(name="data", bufs=1) as dp:
            wb = wp.tile([128, _BW], F32, name="wb")
            nc.sync.dma_start(out=wb, in_=wb_d[:, :])

            def W(nm):
                r, c, off, b = _OFFS[nm]
                return wb[b:b + r, off:off + c]

            # persistent per-half tiles (reused by both halves; only the
            # pad columns must stay zero, and nothing ever writes them)
            zinA = dp.tile([97, HB, T], F32, name="zinA")
            zinB = dp.tile([97, HB, T], F32, name="zinB")
            ZC = dp.tile([128, HB, T], F32, name="ZC")
            SGt = dp.tile([33, 2, HB, SEG], F32, name="SGt")  # sfg | sog
            SIG = dp.tile([33, HB, T], F32, name="SIG")
            TIA = dp.tile([33, HB, T], F32, name="TIA")
            S2T = dp.tile([33, HB, SEG], F32, name="S2T")
            CT = dp.tile([33, HB, SEG], F32, name="CT")
            TC = dp.tile([33, HB, T], F32, name="TC")
            HT = dp.tile([128, HB, SEG], F32, name="HT")
            HL = dp.tile([33, HB, T], F32, name="HL")
            Ff1 = dp.tile([18, HB, T], F32, name="Ff1")
            Ff2 = dp.tile([18, HB, T], F32, name="Ff2")
            Fti = dp.tile([18, HB, T], F32, name="Fti")
            Dg = dp.tile([18, HB, T], F32, name="Dg")
            Gg = dp.tile([18, HB, T], F32, name="Gg")

            nc.vector.memset(SGt, 0.0)
            nc.vector.memset(S2T, 0.0)
            nc.vector.memset(ZC, 0.0)

            sfg_flat = SGt.rearrange("p g s c -> p g (s c)")[0:33, 0, 0:NH]
            s2_flat = S2T.rearrange("p s c -> p (s c)")
            ct_flat = CT.rearrange("p s c -> p (s c)")

            xt_r = xt_d.rearrange("(c p) n -> p c n", p=128)

            for half in range(2):
                hc0 = half * HB * T  # first input column of this half
                # sweep 1 exploits HT == 0 (gate matmuls skipped entirely)
                nc.vector.memset(HT, 0.0)
                with tc.tile_pool(name="xp", bufs=1) as xp:
                    xt_sb = xp.tile([128, 4, HB * T], F32, name="xt_sb")
                    nc.sync.dma_start(
                        out=xt_sb, in_=xt_r[:, :, hc0:hc0 + HB * T])

                    # ---- phase A: input projections -> zinA/zinB/zinC ----
                    with tc.tile_pool(name="pa", bufs=1, space="PSUM") as pa:
                        pg = pa.tile([97, 2, HB, T], F32, name="pg")
                        for gi, gname in ((0, "A"), (1, "B")):
                            for s in range(HB):
                                for k in range(4):
                                    nc.tensor.matmul(
                                        pg[0:97, gi, s, :],
                                        W(f"pa{gname}{k}")[:, 0:97],
                                        xt_sb[:, k, s * T:(s + 1) * T],
                                        start=(k == 0), stop=(k == 3))
                        nc.scalar.activation(zinA, pg[0:97, 0, :, :],
                                             AF.Identity, bias=W("bA")[:, 0:1])
                        nc.scalar.activation(zinB, pg[0:97, 1, :, :],
                                             AF.Identity, bias=W("bB")[:, 0:1])
                    with tc.tile_pool(name="pc", bufs=1, space="PSUM") as pc:
                        pgc = pc.tile([82, HB, T], F32, name="pgc")
                        for s in range(HB):
                            for k in range(4):
                                nc.tensor.matmul(
                                    pgc[0:82, s, :], W(f"paC{k}")[:, 0:82],
                                    xt_sb[:, k, s * T:(s + 1) * T],
                                    start=(k == 0), stop=(k == 3))
                        nc.scalar.activation(ZC[0:82, :, :], pgc,
                                             AF.Identity,
                                             bias=W("bC")[:, 0:1])

                # ---- Picard sweeps ----
                for sw in range(NSWEEPS * sweep_reps):
                    # gates
                    if sw == 0:
                        # HT == 0: gate preacts are just zinA/zinB
                        nc.scalar.activation(SGt[0:33, 0, 0:HB, 1:SEG],
                                             zinA[0:33, :, :], AF.Sigmoid)
                        nc.scalar.activation(SGt[0:33, 1, 0:HB, 1:SEG],
                                             zinB[0:33, :, :], AF.Sigmoid)
                        nc.scalar.activation(SIG, zinA[64:97, :, :],
                                             AF.Sigmoid)
                        nc.scalar.activation(TIA, zinB[64:97, :, :], AF.Tanh)
                    else:
                        with tc.tile_pool(name="pq", bufs=1,
                                          space="PSUM") as pq:
                            gt = pq.tile([97, 2, HB, T], F32, name="gt")
                            for s in range(HB):
                                nc.tensor.matmul(gt[0:97, 0, s, :], W("I97"),
                                                 zinA[0:97, s, :],
                                                 start=True, stop=False)
                                nc.tensor.matmul(gt[0:97, 0, s, :], W("whA"),
                                                 HT[0:67, s, 0:T],
                                                 start=False, stop=True)
                                nc.tensor.matmul(gt[0:97, 1, s, :], W("I97"),
                                                 zinB[0:97, s, :],
                                                 start=True, stop=False)
                                nc.tensor.matmul(gt[0:97, 1, s, :], W("whB"),
                                                 HT[0:67, s, 0:T],
                                                 start=False, stop=True)
                            nc.scalar.activation(SGt[0:33, 0:2, 0:HB, 1:SEG],
                                                 gt[0:33, 0:2, :, :],
                                                 AF.Sigmoid)
                            nc.scalar.activation(SIG, gt[64:97, 0, :, :],
                                                 AF.Sigmoid)
                            nc.scalar.activation(TIA, gt[64:97, 1, :, :],
                                                 AF.Tanh)
                    nc.vector.tensor_mul(S2T[0:33, 0:HB, 1:SEG], SIG, TIA)
                    nc.vector.tensor_tensor_scan(
                        ct_flat, sfg_flat, s2_flat, 0.0, ALU.mult, ALU.add)
                    nc.scalar.activation(TC, CT[0:33, 0:HB, 1:SEG], AF.Tanh)
                    nc.vector.tensor_mul(HL, TC, SGt[0:33, 1, 0:HB, 1:SEG])
                    nc.scalar.activation(ZC[96:114, :, :], HL[0:18, :, :],
                                         AF.Copy)
                    nc.scalar.activation(HT[96:128, 0:HB, 1:SEG],
                                         HL[0:32, :, :], AF.Copy)

                    # CfC layers
                    for lay in range(3):
                        with tc.tile_pool(name="pl", bufs=1,
                                          space="PSUM") as pl:
                            lt = pl.tile([82, HB, T], F32, name="lt")
                            for s in range(HB):
                                if lay == 0:
                                    nc.tensor.matmul(
                                        lt[0:82, s, :], W("L0m"),
                                        ZC[0:114, s, :],
                                        start=True, stop=True)
                                elif lay == 1:
                                    nc.tensor.matmul(
                                        lt[0:76, s, :], W("L1m"),
                                        HT[0:128, s, 1:SEG],
                                        start=True, stop=True)
                                else:
                                    nc.tensor.matmul(
                                        lt[0:67, s, :], W("P2h1"),
                                        HT[32:44, s, 1:SEG],
                                        start=True, stop=False)
                                    nc.tensor.matmul(
                                        lt[0:67, s, :], W("P2h"),
                                        HL[0:33, s, :],
                                        start=False, stop=True)
                            k = (INTER, COMMAND, MOTOR)[lay]
                            bnm = (None, "b1", "b2")[lay]
                            bias = (lambda a, b: W(bnm)[a:b, 0:1]) if bnm \
                                else (lambda a, b: 0.0)
                            nc.scalar.activation(Ff1[0:k, :, :],
                                                 lt[0:k, :, :], AF.Tanh,
                                                 bias=bias(0, k))
                            nc.scalar.activation(Ff2[0:k, :, :],
                                                 lt[32:32 + k, :, :], AF.Tanh,
                                                 bias=bias(32, 32 + k))
                            nc.scalar.activation(Fti[0:k, :, :],
                                                 lt[64:64 + k, :, :],
                                                 AF.Sigmoid,
                                                 bias=bias(64, 64 + k))
                        nc.vector.tensor_sub(Dg[0:k, :, :], Ff2[0:k, :, :],
                                             Ff1[0:k, :, :])
                        nc.vector.tensor_mul(Gg[0:k, :, :], Fti[0:k, :, :],
                                             Dg[0:k, :, :])
                        hrow = (0, 32, 64)[lay]
                        nc.vector.tensor_add(
                            HT[hrow:hrow + k, 0:HB, 1:SEG],
                            Ff1[0:k, :, :], Gg[0:k, :, :])

                # ---- output: h2 trajectory lives at HT[64:67] ----
                nc.sync.dma_start(
                    out=out_d[:, half * HB:(half + 1) * HB, :],
                    in_=HT[64:67, 0:HB, 1:SEG])
    nc.compile()
    return nc


def host_prep(inputs, T=T_FULL):
    x = np.asarray(inputs["x"], np.float32)
    w = prep_weights(inputs)
    in_maps = []
    for i in range(N_CORES):
        xs = x[i * BS:(i + 1) * BS, :T, :]                  # (BS, T, 512)
        xt = np.ascontiguousarray(
            xs.transpose(2, 0, 1).reshape(IN_DIM, BS * T))  # (512, b*T+t)
        m = {"xt": xt}
        m.update(w)
        in_maps.append(m)
    return in_maps


def gather_output(results, T=T_FULL):
    outs = []
    for i in range(N_CORES):
        o = np.asarray(results[i]["out"])                   # (3, BS, T)
        outs.append(o.transpose(1, 2, 0))                   # (BS, T, 3)
    return np.concatenate(outs, axis=0)


_PROGRAM_CACHE = {}


def kernel(**inputs):
    T = T_FULL
    if T not in _PROGRAM_CACHE:
        _PROGRAM_CACHE[T] = build_program(T)
    nc = _PROGRAM_CACHE[T]
    in_maps = host_prep(inputs, T)
    res = run_bass_kernel_spmd(nc, in_maps, list(range(N_CORES)))
    return gather_output(res.results, T)


# revision 11
# speedup vs baseline: 10.4983x; 10.4983x over previous
"""Trainium2 Bass kernel for the DinMod LSTM+CfC (NCP) recurrent network.

Parallel-in-time Picard iteration. The graded execute path costs ~45us
per emitted instruction (fully serialized, size-independent), so instead
of an exact T=512 sequential scan (~16k instructions), we iterate the
whole trajectory: guess h[t]=0, then each sweep recomputes all T steps
with BATCHED instructions (matmuls/ACT/DVE over 4 sequences x 512 steps
at once). The LSTM c-recurrence is linear given the gates, so ONE
tensor_tensor_scan instruction solves it exactly along the time axis per
sweep. The step map is a strong contraction (~10x error reduction per
sweep, measured on the actual weights): 6 sweeps reach ~3e-6 relative
error in fp64 (tolerance is 2e-2).

Per core: 8 sequences (batch elems), processed as 2 independent halves
of 4 (SBUF budget). Column layout per half: 4 segments of 513 columns
(1 zero pad + 512 steps); col(s, t) = s*513 + 1 + t. Trajectory tiles
read at offset 0 give h[t-1] (shifted), offset 1 gives h[t]; the pad
column doubles as the zero initial state and as the scan reset (the
sfg multiplier and S2 addend are 0 there, so the running c state resets
across segment boundaries in the single flattened scan).

Per sweep per half (~65 instructions):
  gates: 4 segs x (inject zinA | whA@h | inject zinB | whB@h) = 16 mm
         -> sigmoid(fg+1)/sigmoid(og) (1 ACT over both groups),
            sigmoid(ig), tanh(ia) [+1 folded into the zinA bias]
  c:     S2 = sig*tia (DVE), c = tensor_tensor_scan(sfg, S2),
         tanh(c) (ACT), h_lstm = tc*sog (DVE)
  CfC l: 2 mm/seg (inject or h0/h1-part + recurrent part), 3 ACT
         (tanh f1, tanh f2, sigmoid ti), 3 DVE (D=f2-f1, G=ti*D,
         h_l = f1+G -> written time-shifted into the H tile)

Verifier constraints honored: all access patterns start at partition
0/32/64/96; dual-SBUF vector operands have equal base partitions
(mixed SBUF/PSUM exempt); PSUM accumulation groups are start/stop pairs
within one bank.
"""

import numpy as np

import concourse.bass as bass
import concourse.mybir as mybir
from concourse import bacc
from concourse.tile import TileContext
from concourse.bass_utils import run_bass_kernel_spmd

IN_DIM, LATENT = 512, 256
INTER, COMMAND, MOTOR = 18, 12, 3
STATE = INTER + COMMAND + MOTOR  # 33
B, T_FULL, N_CORES = 64, 512, 8
BS = B // N_CORES  # 8 sequences per core
HB = 4             # sequences per half
SEG = T_FULL + 1   # 513 padded columns per sequence
NH = HB * SEG      # 2052 columns per half
NSWEEPS = 3

F32 = mybir.dt.float32
AF = mybir.ActivationFunctionType
ALU = mybir.AluOpType

# ---------------------------------------------------------------------------
# Weight blob: every lhsT/bias lives at rows 0:r, cols off:off+c of a single
# [128, BW] tensor (one DMA; base-0 quadrant-legal slices).
# ---------------------------------------------------------------------------
# (name, rows, cols, base_row): base_row shifts the block down so the
# lhsT slice's base partition matches its rhs (matmul requires equality).
_BLOCKS = [
    ("I97", 97, 97, 0), ("I82", 82, 82, 0),
    ("whA", 67, 97, 0), ("whB", 67, 97, 0),
    ("L0m", 114, 82, 0), ("L1m", 128, 76, 0),
    ("P2h1", 12, 67, 32), ("P2h", 33, 67, 0),
    ("bA", 97, 1, 0), ("bB", 97, 1, 0), ("bC", 82, 1, 0),
    ("b1", 76, 1, 0), ("b2", 67, 1, 0),
] + [(f"pa{g}{k}", 128, 97 if g in "AB" else 82, 0)
     for g in "ABC" for k in range(4)]

_OFFS = {}
_BW = 0
for _nm, _r, _c, _b in _BLOCKS:
    _OFFS[_nm] = (_r, _c, _BW, _b)
    _BW += _c

# HT (h trajectory) row layout: h0@0:18, h1@32:44, h2@64:67 (quadrant-legal
# write starts for the three per-layer h writers); unused rows stay 0.
_HTROW = [j for j in range(18)] + [32 + j for j in range(12)] + [64 + j for j in range(3)]


def prep_weights(inp):
    g = {k: np.asarray(v, np.float64) for k, v in inp.items()}
    Wf, bf = g["fc1_w"], g["fc1_b"]            # (256,512), (256,)
    wi, bi, wh = g["lstm_wi"], g["lstm_bi"], g["lstm_wh"]
    ia, ig, fg, og = (slice(0, 33), slice(33, 66),
                      slice(66, 99), slice(99, 132))

    blob = np.zeros((128, _BW), np.float64)

    def put(nm, m):
        r, c, off, b = _OFFS[nm]
        assert m.shape == (r, c), (nm, m.shape, (r, c))
        blob[b:b + r, off:off + c] = m

    put("I97", np.eye(97))
    put("I82", np.eye(82))

    # recurrent gate contractions: out rows (grp A) fg@0:33, ig@64:97;
    # (grp B) og@0:33, ia@64:97. Contract dim = HT rows (h comp j at _HTROW[j]).
    def wh_block(lo_sl, hi_sl):
        m = np.zeros((67, 97))
        for j in range(STATE):
            r = _HTROW[j]
            m[r, 0:33] = wh[lo_sl, j]
            m[r, 64:97] = wh[hi_sl, j]
        return m

    put("whA", wh_block(fg, ig))
    put("whB", wh_block(og, ia))

    # CfC layer weights (masked), ti = sigmoid((ta+tb) @ xc + (tab+tbb))
    w1m, w2m, wab, bab = [], [], [], []
    for l in range(3):
        w1m.append(g[f"ff1w{l}"] * g[f"mask{l}"])
        w2m.append(g[f"ff2w{l}"] * g[f"mask{l}"])
        wab.append(g[f"taw{l}"] + g[f"tbw{l}"])
        bab.append(g[f"tab{l}"] + g[f"tbb{l}"])

    # layer 0, one matmul: rhs = ZC (zinC@0:82, hs0 copy@96:114); out rows
    # f1@0:18, f2@32:50, ti@64:82
    m = np.zeros((114, 82))
    m[0:82, 0:82] = np.eye(82)
    for j in range(INTER):
        m[96 + j, 0:18] = w1m[0][:, LATENT + j]
        m[96 + j, 32:50] = w2m[0][:, LATENT + j]
        m[96 + j, 64:82] = wab[0][:, LATENT + j]
    put("L0m", m)

    # layer 1, one matmul: rhs = HT (h0@0:18, hl copy@96:128 -> hs1@114:126);
    # out rows f1@0:12, f2@32:44, ti@64:76
    m = np.zeros((128, 76))
    for j in range(INTER):
        m[j, 0:12] = w1m[1][:, j]
        m[j, 32:44] = w2m[1][:, j]
        m[j, 64:76] = wab[1][:, j]
    for j in range(COMMAND):
        m[114 + j, 0:12] = w1m[1][:, INTER + j]
        m[114 + j, 32:44] = w2m[1][:, INTER + j]
        m[114 + j, 64:76] = wab[1][:, INTER + j]
    put("L1m", m)
    bs1 = np.zeros((76, 1))
    bs1[0:12, 0] = g["ff1b1"]
    bs1[32:44, 0] = g["ff2b1"]
    bs1[64:76, 0] = bab[1]
    put("b1", bs1)

    # layer 2: xc = [h1(12), hs2(3)]; out rows f1@0:3, f2@32:35, ti@64:67
    m = np.zeros((12, 67))
    for j in range(COMMAND):
        m[j, 0:3] = w1m[2][:, j]
        m[j, 32:35] = w2m[2][:, j]
        m[j, 64:67] = wab[2][:, j]
    put("P2h1", m)
    m = np.zeros((33, 67))
    for j in range(MOTOR):
        m[30 + j, 0:3] = w1m[2][:, COMMAND + j]
        m[30 + j, 32:35] = w2m[2][:, COMMAND + j]
        m[30 + j, 64:67] = wab[2][:, COMMAND + j]
    put("P2h", m)
    bs2 = np.zeros((67, 1))
    bs2[0:3, 0] = g["ff1b2"]
    bs2[32:35, 0] = g["ff2b2"]
    bs2[64:67, 0] = bab[2]
    put("b2", bs2)

    # phase A composed input projections (fc1 folded in); +1.0 on the fg
    # bias so the sigmoid ACT needs no extra bias.
    WA = np.zeros((97, IN_DIM)); bA = np.zeros((97, 1))
    WA[0:33] = wi[fg] @ Wf
    bA[0:33, 0] = wi[fg] @ bf + bi[fg] + 1.0
    WA[64:97] = wi[ig] @ Wf
    bA[64:97, 0] = wi[ig] @ bf + bi[ig]
    WB = np.zeros((97, IN_DIM)); bB = np.zeros((97, 1))
    WB[0:33] = wi[og] @ Wf
    bB[0:33, 0] = wi[og] @ bf + bi[og]
    WB[64:97] = wi[ia] @ Wf
    bB[64:97, 0] = wi[ia] @ bf + bi[ia]
    WC = np.zeros((82, IN_DIM)); bC = np.zeros((82, 1))
    WC[0:18] = w1m[0][:, 0:LATENT] @ Wf
    bC[0:18, 0] = w1m[0][:, 0:LATENT] @ bf + g["ff1b0"]
    WC[32:50] = w2m[0][:, 0:LATENT] @ Wf
    bC[32:50, 0] = w2m[0][:, 0:LATENT] @ bf + g["ff2b0"]
    WC[64:82] = wab[0][:, 0:LATENT] @ Wf
    bC[64:82, 0] = wab[0][:, 0:LATENT] @ bf + bab[0]
    put("bA", bA)
    put("bB", bB)
    put("bC", bC)
    for gname, W in (("A", WA), ("B", WB), ("C", WC)):
        for k in range(4):
            put(f"pa{gname}{k}",
                np.ascontiguousarray(W[:, 128 * k:128 * (k + 1)].T))

    return {"wblob": blob.astype(np.float32)}


def build_program(T=T_FULL, opts=()):
    assert T == T_FULL
    opts = set(opts)
    sweep_reps = 1
    for o in opts:
        if isinstance(o, str) and o.startswith("reps"):
            sweep_reps = int(o[4:])

    nc = bacc.Bacc("TRN2")
    xt_d = nc.dram_tensor("xt", [IN_DIM, BS * T], F32, kind="ExternalInput")
    wb_d = nc.dram_tensor("wblob", [128, _BW], F32, kind="ExternalInput")
    out_d = nc.dram_tensor("out", [MOTOR, BS, T], F32, kind="ExternalOutput")

    with TileContext(nc) as tc:
        with tc.tile_pool(name="wpool", bufs=1) as wp, \
             tc.tile_pool(name="data", bufs=1) as dp:
            wb = wp.tile([128, _BW], F32, name="wb")
            nc.sync.dma_start(out=wb, in_=wb_d[:, :])

            def W(nm):
                r, c, off, b = _OFFS[nm]
                return wb[b:b + r, off:off + c]

            # persistent per-half tiles (reused by both halves; only the
            # pad columns must stay zero, and nothing ever writes them)
            zinA = dp.tile([97, HB, T], F32, name="zinA")
            zinB = dp.tile([97, HB, T], F32, name="zinB")
            ZC = dp.tile([128, HB, T], F32, name="ZC")
            SGt = dp.tile([33, 2, HB, SEG], F32, name="SGt")  # sfg | sog
            SIG = dp.tile([33, HB, T], F32, name="SIG")
            TIA = dp.tile([33, HB, T], F32, name="TIA")
            S2T = dp.tile([33, HB, SEG], F32, name="S2T")
            CT = dp.tile([33, HB, SEG], F32, name="CT")
            TC = dp.tile([33, HB, T], F32, name="TC")
            HT = dp.tile([128, HB, SEG], F32, name="HT")
            HL = dp.tile([33, HB, T], F32, name="HL")
            Ff1 = dp.tile([18, HB, T], F32, name="Ff1")
            Ff2 = dp.tile([18, HB, T], F32, name="Ff2")
            Fti = dp.tile([18, HB, T], F32, name="Fti")
            Dg = dp.tile([18, HB, T], F32, name="Dg")
            Gg = dp.tile([18, HB, T], F32, name="Gg")

            nc.vector.memset(SGt, 0.0)
            nc.vector.memset(S2T, 0.0)
            nc.vector.memset(ZC, 0.0)

            sfg_flat = SGt.rearrange("p g s c -> p g (s c)")[0:33, 0, 0:NH]
            s2_flat = S2T.rearrange("p s c -> p (s c)")
            ct_flat = CT.rearrange("p s c -> p (s c)")

            xt_r = xt_d.rearrange("(c p) n -> p c n", p=128)

            for half in range(2):
                hc0 = half * HB * T  # first input column of this half
                # sweep 1 exploits HT == 0 (gate matmuls skipped entirely)
                nc.vector.memset(HT, 0.0)
                with tc.tile_pool(name="xp", bufs=1) as xp:
                    xt_sb = xp.tile([128, 4, HB * T], F32, name="xt_sb")
                    nc.sync.dma_start(
                        out=xt_sb, in_=xt_r[:, :, hc0:hc0 + HB * T])

                    # ---- phase A: input projections -> zinA/zinB/zinC ----
                    with tc.tile_pool(name="pa", bufs=1, space="PSUM") as pa:
                        pg = pa.tile([97, 2, HB, T], F32, name="pg")
                        for gi, gname in ((0, "A"), (1, "B")):
                            for s in range(HB):
                                for k in range(4):
                                    nc.tensor.matmul(
                                        pg[0:97, gi, s, :],
                                        W(f"pa{gname}{k}")[:, 0:97],
                                        xt_sb[:, k, s * T:(s + 1) * T],
                                        start=(k == 0), stop=(k == 3))
                        nc.scalar.activation(zinA, pg[0:97, 0, :, :],
                                             AF.Identity, bias=W("bA")[:, 0:1])
                        nc.scalar.activation(zinB, pg[0:97, 1, :, :],
                                             AF.Identity, bias=W("bB")[:, 0:1])
                    with tc.tile_pool(name="pc", bufs=1, space="PSUM") as pc:
                        pgc = pc.tile([82, HB, T], F32, name="pgc")
                        for s in range(HB):
                            for k in range(4):
                                nc.tensor.matmul(
                                    pgc[0:82, s, :], W(f"paC{k}")[:, 0:82],
                                    xt_sb[:, k, s * T:(s + 1) * T],
                                    start=(k == 0), stop=(k == 3))
                        nc.scalar.activation(ZC[0:82, :, :], pgc,
                                             AF.Identity,
                                             bias=W("bC")[:, 0:1])

                # ---- Picard sweeps ----
                for sw in range(NSWEEPS * sweep_reps):
                    # gates
                    if sw == 0:
                        # HT == 0: gate preacts are just zinA/zinB
                        nc.scalar.activation(SGt[0:33, 0, 0:HB, 1:SEG],
                                             zinA[0:33, :, :], AF.Sigmoid)
                        nc.scalar.activation(SGt[0:33, 1, 0:HB, 1:SEG],
                                             zinB[0:33, :, :], AF.Sigmoid)
                        nc.scalar.activation(SIG, zinA[64:97, :, :],
                                             AF.Sigmoid)
                        nc.scalar.activation(TIA, zinB[64:97, :, :], AF.Tanh)
                    else:
                        with tc.tile_pool(name="pq", bufs=1,
                                          space="PSUM") as pq:
                            gt = pq.tile([97, 2, HB, T], F32, name="gt")
                            for s in range(HB):
                                nc.tensor.matmul(gt[0:97, 0, s, :], W("I97"),
                                                 zinA[0:97, s, :],
                                                 start=True, stop=False)
                                nc.tensor.matmul(gt[0:97, 0, s, :], W("whA"),
                                                 HT[0:67, s, 0:T],
                                                 start=False, stop=True)
                                nc.tensor.matmul(gt[0:97, 1, s, :], W("I97"),
                                                 zinB[0:97, s, :],
                                                 start=True, stop=False)
                                nc.tensor.matmul(gt[0:97, 1, s, :], W("whB"),
                                                 HT[0:67, s, 0:T],
                                                 start=False, stop=True)
                            nc.scalar.activation(SGt[0:33, 0:2, 0:HB, 1:SEG],
                                                 gt[0:33, 0:2, :, :],
                                                 AF.Sigmoid)
                            nc.scalar.activation(SIG, gt[64:97, 0, :, :],
                                                 AF.Sigmoid)
                            nc.scalar.activation(TIA, gt[64:97, 1, :, :],
                                                 AF.Tanh)
                    nc.vector.tensor_mul(S2T[0:33, 0:HB, 1:SEG], SIG, TIA)
                    nc.vector.tensor_tensor_scan(
                        ct_flat, sfg_flat, s2_flat, 0.0, ALU.mult, ALU.add)
                    nc.scalar.activation(TC, CT[0:33, 0:HB, 1:SEG], AF.Tanh)
                    nc.vector.tensor_mul(HL, TC, SGt[0:33, 1, 0:HB, 1:SEG])
                    nc.scalar.activation(ZC[96:114, :, :], HL[0:18, :, :],
                                         AF.Copy)
                    nc.scalar.activation(HT[96:128, 0:HB, 1:SEG],
                                         HL[0:32, :, :], AF.Copy)

                    # CfC layers
                    for lay in range(3):
                        with tc.tile_pool(name="pl", bufs=1,
                                          space="PSUM") as pl:
                            lt = pl.tile([82, HB, T], F32, name="lt")
                            for s in range(HB):
                                if lay == 0:
                                    nc.tensor.matmul(
                                        lt[0:82, s, :], W("L0m"),
                                        ZC[0:114, s, :],
                                        start=True, stop=True)
                                elif lay == 1:
                                    nc.tensor.matmul(
                                        lt[0:76, s, :], W("L1m"),
                                        HT[0:128, s, 1:SEG],
                                        start=True, stop=True)
                                else:
                                    nc.tensor.matmul(
                                        lt[0:67, s, :], W("P2h1"),
                                        HT[32:44, s, 1:SEG],
                                        start=True, stop=False)
                                    nc.tensor.matmul(
                                        lt[0:67, s, :], W("P2h"),
                                        HL[0:33, s, :],
                                        start=False, stop=True)
                            k = (INTER, COMMAND, MOTOR)[lay]
                            bnm = (None, "b1", "b2")[lay]
                            bias = (lambda a, b: W(bnm)[a:b, 0:1]) if bnm \
                                else (lambda a, b: 0.0)
                            nc.scalar.activation(Ff1[0:k, :, :],
                                                 lt[0:k, :, :], AF.Tanh,
                                                 bias=bias(0, k))
                            nc.scalar.activation(Ff2[0:k, :, :],
                                                 lt[32:32 + k, :, :], AF.Tanh,
                                                 bias=bias(32, 32 + k))
                            nc.scalar.activation(Fti[0:k, :, :],
                                                 lt[64:64 + k, :, :],
                                                 AF.Sigmoid,
                                                 bias=bias(64, 64 + k))
                        nc.vector.tensor_sub(Dg[0:k, :, :], Ff2[0:k, :, :],
                                             Ff1[0:k, :, :])
                        nc.vector.tensor_mul(Gg[0:k, :, :], Fti[0:k, :, :],
                                             Dg[0:k, :, :])
                        hrow = (0, 32, 64)[lay]
                        nc.vector.tensor_add(
                            HT[hrow:hrow + k, 0:HB, 1:SEG],
                            Ff1[0:k, :, :], Gg[0:k, :, :])

                # ---- output: h2 trajectory lives at HT[64:67] ----
                nc.sync.dma_start(
                    out=out_d[:, half * HB:(half + 1) * HB, :],
                    in_=HT[64:67, 0:HB, 1:SEG])
    nc.compile()
    return nc


def host_prep(inputs, T=T_FULL):
    x = np.asarray(inputs["x"], np.float32)
    w = prep_weights(inputs)
    in_maps = []
    for i in range(N_CORES):
        xs = x[i * BS:(i + 1) * BS, :T, :]                  # (BS, T, 512)
        xt = np.ascontiguousarray(
            xs.transpose(2, 0, 1).reshape(IN_DIM, BS * T))  # (512, b*T+t)
        m = {"xt": xt}
        m.update(w)
        in_maps.append(m)
    return in_maps


def gather_output(results, T=T_FULL):
    outs = []
    for i in range(N_CORES):
        o = np.asarray(results[i]["out"])                   # (3, BS, T)
        outs.append(o.transpose(1, 2, 0))                   # (BS, T, 3)
    return np.concatenate(outs, axis=0)


_PROGRAM_CACHE = {}


def kernel(**inputs):
    T = T_FULL
    if T not in _PROGRAM_CACHE:
        _PROGRAM_CACHE[T] = build_program(T)
    nc = _PROGRAM_CACHE[T]
    in_maps = host_prep(inputs, T)
    res = run_bass_kernel_spmd(nc, in_maps, list(range(N_CORES)))
    return gather_output(res.results, T)
